# revision 23
# baseline (speedup 1.0000x reference)
"""DifferentialAttention on 8 TRN2 NeuronCores.

Sharding: tensor-parallel over heads (2 heads per core), no device
collectives. Each core computes qkv for its heads, causal differential
attention + per-head LayerNorm, and a partial output projection through
its slice of W_o columns; the host sums the 8 partial outputs.

All matmuls run as float32r (fp22 mantissa, full PE rate at N>=256).
"""

import numpy as np

HEAD_DIM = 64
N_HEADS = 16
D_MODEL = 2048
SEQ = 2048
LAYER_IDX = 12
LN_EPS = 1e-5
N_CORES = 8
HPC = N_HEADS // N_CORES          # heads per core = 2
CHUNK = 512                       # sq chunk width
NCHUNK = SEQ // CHUNK             # 4
NDT = D_MODEL // 128              # 16 d-tiles
NST = SEQ // 128                  # 16 s-tiles

_SYNC_CNT = [0]


def _patch_tile_drain(tile_mod, bass_rust):
    """The walrus build in this container encodes at most one sem wait per
    instruction; TileContext's exit drain carries one wait per producer
    proc. Split the extras onto single-wait NOPs."""
    from concourse.vector_clock import ScopedClock

    def patched(self, tick_clock, wait_clock):
        nc = self.nc
        drain_inst = nc.sync.drain()
        wait_clock.add_sem_waits(
            drain_inst.ins, ScopedClock({None: tick_clock.global_clock})
        )
        si = drain_inst.ins.sync_info
        waits = list(si.on_wait or [])
        if len(waits) > 1:
            si.on_wait = [waits[0]]
            for w in waits[1:]:
                nop = nc.sync.nop()
                nop.ins.sync_info = bass_rust.SyncInfo(on_wait=[w], on_update=[])
        nc.all_engine_barrier()
        popped = nc._tile_sem_poison_stack.pop()
        assert popped is self._sem_poison
        nc.clear_and_free_semaphores(list(self.sems.allocated().values()))
        nc.all_engine_barrier()

    tile_mod.TileContext._drain_and_barrier = patched


def _fix_sync_limits(nc, mybir, bass_rust):
    """Split multi-wait / multi-update instructions into single-wait NOP
    chains on the same engine queue (walrus single-sync-slot limit)."""

    def nop(engine, wait=None, update=None):
        _SYNC_CNT[0] += 1
        n = mybir.InstNoOp(name=f"syncsplit-{_SYNC_CNT[0]}", ins=[], outs=[])
        n.engine = engine
        n.sync_info = bass_rust.SyncInfo(
            on_wait=[wait] if wait is not None else [],
            on_update=[update] if update is not None else [],
        )
        return n

    for f in nc.m.functions:
        for b in f.blocks:
            out = []
            for inst in b.instructions:
                si = inst.sync_info
                post = []
                if si is not None:
                    waits = list(si.on_wait or [])
                    if len(waits) > 1:
                        for w in waits[:-1]:
                            out.append(nop(inst.engine, wait=w))
                        si.on_wait = [waits[-1]]
                    ups = list(si.on_update or [])
                    if len(ups) > 1:
                        si.on_update = [ups[0]]
                        for u in ups[1:]:
                            post.append(nop(inst.engine, update=u))
                out.append(inst)
                out.extend(post)
            b.instructions = out


def _install_ntff_shim():
    """Register the axon NTFF profile hook (used only when tracing)."""
    import sys, types
    if "antenv.axon_hooks" in sys.modules:
        return
    try:
        mod = types.ModuleType("antenv.axon_hooks")
        mod._hook = None
        mod.set_axon_ntff_profile_hook = lambda h: setattr(mod, "_hook", h)
        mod.get_axon_ntff_profile_hook = lambda: mod._hook
        sys.modules["antenv.axon_hooks"] = mod
        import antenv
        antenv.axon_hooks = mod
        from trn_agent_boot.trn_boot import _ntff_profile_via_ctypes
        mod.set_axon_ntff_profile_hook(
            _ntff_profile_via_ctypes("/opt/axon/libaxon_pjrt.so")
        )
    except Exception:
        pass


def _build_nc():
    import bass_rust
    import concourse.bass as bass
    import concourse.tile as tile
    from concourse import mybir

    _patch_tile_drain(tile, bass_rust)

    f32 = mybir.dt.float32
    f32r = mybir.dt.float32r
    bf16 = mybir.dt.bfloat16
    AT = mybir.ActivationFunctionType
    OP = mybir.AluOpType

    nc = bass.Bass()

    xT = nc.dram_tensor("xT", [D_MODEL, SEQ], f32r, kind="ExternalInput")
    wqkT = nc.dram_tensor("wqkT", [D_MODEL, 4 * 128], f32r, kind="ExternalInput")
    wvT = nc.dram_tensor("wvT", [D_MODEL, HPC * 128], f32r, kind="ExternalInput")
    woT = nc.dram_tensor("woT", [HPC * 128, D_MODEL], f32r, kind="ExternalInput")
    lamneg = nc.dram_tensor("lamneg", [128], f32, kind="ExternalInput")
    gamma = nc.dram_tensor("gamma", [HPC, 128], f32, kind="ExternalInput")
    beta = nc.dram_tensor("beta", [HPC, 128], f32, kind="ExternalInput")
    trimask = nc.dram_tensor("trimask", [128, 128], f32r, kind="ExternalInput")
    onesin = nc.dram_tensor("onesin", [128, 128], f32r, kind="ExternalInput")
    meanin = nc.dram_tensor("meanin", [128, 1], f32r, kind="ExternalInput")
    y = nc.dram_tensor("y", [SEQ, D_MODEL], bf16, kind="ExternalOutput")

    with tile.TileContext(nc) as tc, nc.allow_low_precision(reason="fp32r pipeline"):
        import contextlib
        with contextlib.ExitStack() as ctx:
            consts = ctx.enter_context(tc.tile_pool(name="consts", bufs=1))
            main = ctx.enter_context(tc.tile_pool(name="main", bufs=1))
            drp = ctx.enter_context(tc.tile_pool(name="drp", bufs=4, space="DRAM"))

            # ---- constants ----
            lam_sb = consts.tile([128, 1], f32)
            nc.sync.dma_start(lam_sb[:, 0], lamneg[:])
            gam_sb = [consts.tile([128, 1], f32, name=f"gam{h}") for h in range(HPC)]
            bet_sb = [consts.tile([128, 1], f32, name=f"bet{h}") for h in range(HPC)]
            for h in range(HPC):
                nc.sync.dma_start(gam_sb[h][:, 0], gamma[h, :])
                nc.sync.dma_start(bet_sb[h][:, 0], beta[h, :])
            tri_sb = consts.tile([128, 128], f32r)
            nc.sync.dma_start(tri_sb[:], trimask[:])
            ones_col = consts.tile([128, 1], f32r)
            nc.sync.dma_start(ones_col[:], onesin[:, 0:1])
            mean_col = consts.tile([128, 1], f32r)
            nc.sync.dma_start(mean_col[:], meanin[:])

            # ---- persistent activations ----
            qk_sb = [main.tile([128, SEQ], f32r, name=f"qk{i}") for i in range(4)]
            v_sb = [main.tile([128, HPC * 128], f32r, name=f"v{t}") for t in range(NST)]
            w_sb = [main.tile([128, SEQ], f32r, name=f"w{h}") for h in range(HPC)]
            outT_sb = [main.tile([128, SEQ], f32r, name=f"outT{h}") for h in range(HPC)]
            wo_sb = [main.tile([128, SEQ], f32r, name=f"wo{i}") for i in range(HPC)]
            d1_all = [main.tile([1, SEQ], f32, name=f"d1a{h}") for h in range(HPC)]

            def bcast(vec_ap, out_ap, n):
                # [1, n] sbuf -> dram -> [128, n] sbuf partition-broadcast
                s = drp.tile([1, n], f32, tag="dbc")
                nc.sync.dma_start(s[:], vec_ap)
                bap = bass.AP(tensor=s.tensor, offset=s.offset,
                              ap=[[0, 128]] + list(s.ap[1:]))
                nc.sync.dma_start(out_ap, bap)

            # ================= phase 1: qkv projection =================
            with (
                tc.tile_pool(name="p1w", bufs=1) as p1w,
                tc.tile_pool(name="p1x", bufs=18) as p1x,
                tc.tile_pool(name="p1ps", bufs=4, space="PSUM") as p1ps,
            ):
                wqk_t = [p1w.tile([128, 512], f32r, name=f"wqk{d}") for d in range(NDT)]
                wv_t = [p1w.tile([128, HPC * 128], f32r, name=f"wv{d}") for d in range(NDT)]

                for c in range(NCHUNK):
                    xc = []
                    for d in range(NDT):
                        if c == 0:
                            nc.sync.dma_start(
                                wqk_t[d][:], wqkT[128 * d:128 * (d + 1), :])
                        t = p1x.tile([128, CHUNK], f32r, tag="xc")
                        nc.sync.dma_start(
                            t[:], xT[128 * d:128 * (d + 1), CHUNK * c:CHUNK * (c + 1)])
                        xc.append(t)
                        if c == 0:
                            nc.sync.dma_start(
                                wv_t[d][:], wvT[128 * d:128 * (d + 1), :])
                    if c == 1:
                        for i in range(HPC):
                            nc.sync.dma_start(
                                wo_sb[i][:], woT[128 * i:128 * (i + 1), :])
                    # d-outer accumulation: compute starts once the first
                    # d-tile of weights+x has landed
                    qps = [p1ps.tile([128, CHUNK], f32, tag="qkps", name=f"qps{ct}")
                           for ct in range(4)]
                    vps = [p1ps.tile([128, HPC * 128], f32, tag="vps", name=f"vps{ss}")
                           for ss in range(4)]
                    for d in range(NDT):
                        for ct in range(4):
                            nc.tensor.matmul(
                                qps[ct][:], wqk_t[d][:, 128 * ct:128 * (ct + 1)],
                                xc[d][:], start=(d == 0), stop=(d == NDT - 1))
                        for ss in range(4):
                            nc.tensor.matmul(
                                vps[ss][:], xc[d][:, 128 * ss:128 * (ss + 1)],
                                wv_t[d][:], start=(d == 0), stop=(d == NDT - 1))
                    for ct in range(4):
                        nc.vector.tensor_copy(
                            qk_sb[ct][:, CHUNK * c:CHUNK * (c + 1)], qps[ct][:])
                    for ss in range(4):
                        nc.vector.tensor_copy(v_sb[4 * c + ss][:], vps[ss][:])

            # ===== phase 2: attention + LN + projection, chunk-pipelined =====
            with (
                tc.tile_pool(name="p2e", bufs=2, space="PSUM") as p2e,
                tc.tile_pool(name="p2a1", bufs=2, space="PSUM") as p2a1,
                tc.tile_pool(name="p2a2", bufs=1, space="PSUM") as p2a2,
                tc.tile_pool(name="p2d", bufs=1, space="PSUM") as p2d,
                tc.tile_pool(name="p2y", bufs=1, space="PSUM") as p2y,
                tc.tile_pool(name="p2sb", bufs=3) as p2sb,
                tc.tile_pool(name="p2t", bufs=2) as p2t,
                tc.tile_pool(name="p2s", bufs=8) as p2s,
                tc.tile_pool(name="p2ys", bufs=4) as p2ys,
            ):
                for h in range(HPC):
                    qT = qk_sb[h]
                    kT = qk_sb[2 + h]
                    for c in range(NCHUNK):
                        n_sk = 4 * (c + 1)
                        csl = slice(CHUNK * c, CHUNK * (c + 1))
                        a1 = p2a1.tile([128, CHUNK], f32, tag="a1")
                        a2 = p2a2.tile([128, CHUNK], f32, tag="a2")
                        d1 = p2d.tile([1, CHUNK], f32, tag="d1")
                        d2 = p2d.tile([1, CHUNK], f32, tag="d2")
                        for t in range(n_sk):
                            diag = t >= 4 * c
                            f0 = 128 * (t - 4 * c) if diag else 0
                            sl = slice(f0, CHUNK)
                            qsl = slice(CHUNK * c + f0, CHUNK * (c + 1))
                            e1p = p2e.tile([128, CHUNK], f32, tag="e")
                            e2p = p2e.tile([128, CHUNK], f32, tag="e")
                            nc.tensor.matmul(
                                e1p[:, sl], kT[0:64, 128 * t:128 * (t + 1)],
                                qT[0:64, qsl], start=True, stop=True)
                            nc.tensor.matmul(
                                e2p[:, sl], kT[64:128, 128 * t:128 * (t + 1)],
                                qT[64:128, qsl], start=True, stop=True)
                            e1 = p2sb.tile([128, CHUNK], f32r, tag="e1")
                            e2 = p2sb.tile([128, CHUNK], f32r, tag="e2")
                            nc.scalar.activation(e1[:, sl], e1p[:, sl], AT.Exp)
                            nc.scalar.activation(e2[:, sl], e2p[:, sl], AT.Exp)
                            if diag:
                                dsl = slice(f0, f0 + 128)
                                nc.vector.tensor_tensor(
                                    e1[:, dsl], e1[:, dsl], tri_sb[:], OP.mult)
                                nc.vector.tensor_tensor(
                                    e2[:, dsl], e2[:, dsl], tri_sb[:], OP.mult)
                            first, last = (t == 0), (t == n_sk - 1)
                            vt = v_sb[t][:, 128 * h:128 * (h + 1)]
                            nc.tensor.matmul(a1[:, sl], vt, e1[:, sl],
                                             start=first, stop=last)
                            nc.tensor.matmul(a2[:, sl], vt, e2[:, sl],
                                             start=first, stop=last)
                            nc.tensor.matmul(d1[:, sl], ones_col[:], e1[:, sl],
                                             start=first, stop=last)
                            nc.tensor.matmul(d2[:, sl], ones_col[:], e2[:, sl],
                                             start=first, stop=last)
                        # differential combine via LayerNorm scale-invariance:
                        # w' = A1u - (d1/d2)*lam*A2u
                        nc.vector.tensor_copy(d1_all[h][:, csl], d1[:])
                        rd2 = p2s.tile([1, CHUNK], f32, tag="rd")
                        nc.vector.reciprocal(rd2[:], d2[:])
                        r = p2s.tile([1, CHUNK], f32, tag="rd")
                        nc.vector.tensor_tensor(r[:], d1_all[h][:, csl], rd2[:],
                                                OP.mult)
                        rb = p2t.tile([128, CHUNK], f32, tag="rb")
                        bcast(r[:], rb[:], CHUNK)
                        ta2 = p2t.tile([128, CHUNK], f32, tag="ta2")
                        nc.vector.tensor_tensor(ta2[:], a2[:], rb[:], OP.mult)
                        nc.vector.scalar_tensor_tensor(
                            w_sb[h][:, csl],
                            in0=ta2[:], scalar=lam_sb[:], in1=a1[:],
                            op0=OP.mult, op1=OP.add)
                        # ---- LayerNorm for this chunk ----
                        wsq = p2t.tile([128, CHUNK], f32r, tag="wsq")
                        nc.vector.tensor_tensor(wsq[:], w_sb[h][:, csl],
                                                w_sb[h][:, csl], OP.mult)
                        s1f = p2e.tile([128, CHUNK], f32, tag="e", name="s1f")
                        s2f = p2e.tile([128, CHUNK], f32, tag="e", name="s2f")
                        s1 = s1f[0:1, :]
                        s2 = s2f[0:1, :]
                        nc.tensor.matmul(s1, mean_col[:], w_sb[h][:, csl],
                                         start=True, stop=True)
                        nc.tensor.matmul(s2, mean_col[:], wsq[:],
                                         start=True, stop=True)
                        mu = p2s.tile([1, CHUNK], f32, tag="sm")
                        nc.vector.tensor_copy(mu[:], s1)
                        mu2 = p2s.tile([1, CHUNK], f32, tag="sm")
                        nc.vector.tensor_tensor(mu2[:], mu[:], mu[:], OP.mult)
                        var = p2s.tile([1, CHUNK], f32, tag="sm")
                        nc.vector.tensor_tensor(var[:], s2, mu2[:], OP.subtract)
                        d1sq = p2s.tile([1, CHUNK], f32, tag="sm")
                        nc.vector.tensor_tensor(d1sq[:], d1_all[h][:, csl],
                                                d1_all[h][:, csl], OP.mult)
                        varep = p2s.tile([1, CHUNK], f32, tag="sm")
                        nc.vector.scalar_tensor_tensor(
                            varep[:], in0=d1sq[:], scalar=LN_EPS, in1=var[:],
                            op0=OP.mult, op1=OP.add)
                        # rsqrt via exp(-0.5*ln(x)) — same ACT table set as exp
                        lnv = p2s.tile([1, CHUNK], f32, tag="sm")
                        nc.scalar.activation(lnv[:], varep[:], AT.Ln)
                        rsd = p2s.tile([1, CHUNK], f32, tag="sm")
                        nc.scalar.activation(rsd[:], lnv[:], AT.Exp, scale=-0.5)
                        mrs = p2s.tile([1, CHUNK], f32, tag="sm")
                        nc.vector.tensor_tensor(mrs[:], mu[:], rsd[:], OP.mult)
                        rsd_b = p2t.tile([128, CHUNK], f32, tag="rsdb")
                        bcast(rsd[:], rsd_b[:], CHUNK)
                        mrs_b = p2t.tile([128, CHUNK], f32, tag="mrsb")
                        bcast(mrs[:], mrs_b[:], CHUNK)
                        u1 = p2t.tile([128, CHUNK], f32, tag="u1")
                        nc.vector.tensor_tensor(u1[:], w_sb[h][:, csl], rsd_b[:],
                                                OP.mult)
                        u2 = p2t.tile([128, CHUNK], f32, tag="u2")
                        nc.vector.tensor_tensor(u2[:], u1[:], mrs_b[:],
                                                OP.subtract)
                        nc.vector.tensor_scalar(
                            outT_sb[h][:, csl], u2[:], gam_sb[h][:], bet_sb[h][:],
                            OP.mult, OP.add)
                        # ---- projection for this chunk (both heads ready) ----
                        if h == HPC - 1:
                            for st in range(4 * c, 4 * (c + 1)):
                                ssl = slice(128 * st, 128 * (st + 1))
                                for oc in range(NCHUNK):
                                    osl = slice(CHUNK * oc, CHUNK * (oc + 1))
                                    yp = p2y.tile([128, CHUNK], f32, tag="y")
                                    for i in range(HPC):
                                        nc.tensor.matmul(
                                            yp[:], outT_sb[i][:, ssl],
                                            wo_sb[i][:, osl],
                                            start=(i == 0), stop=(i == HPC - 1))
                                    ys = p2ys.tile([128, CHUNK], bf16, tag="ys")
                                    nc.vector.tensor_copy(ys[:], yp[:])
                                    nc.sync.dma_start(y[ssl, osl], ys[:])

    from concourse import mybir as _mb
    _fix_sync_limits(nc, _mb, bass_rust)
    return nc


_NC_CACHE = {}


def _get_nc():
    if "nc" not in _NC_CACHE:
        _NC_CACHE["nc"] = _build_nc()
    return _NC_CACHE["nc"]


def kernel(x, W_qkv, W_o, lambda_q1, lambda_k1, lambda_q2, lambda_k2,
           gn_gamma, gn_beta):
    import os
    _install_ntff_shim()
    from concourse.bass_utils import run_bass_kernel_spmd

    x = np.asarray(x, np.float32)
    W_qkv = np.asarray(W_qkv, np.float32)
    W_o = np.asarray(W_o, np.float32)
    lambda_q1 = np.asarray(lambda_q1, np.float32)
    lambda_k1 = np.asarray(lambda_k1, np.float32)
    lambda_q2 = np.asarray(lambda_q2, np.float32)
    lambda_k2 = np.asarray(lambda_k2, np.float32)
    gn_gamma = np.asarray(gn_gamma, np.float32)
    gn_beta = np.asarray(gn_beta, np.float32)

    lambda_init = np.float32(0.8 - 0.6 * np.exp(-0.3 * LAYER_IDX))
    lam = (np.exp(lambda_q1 * lambda_k1) - np.exp(lambda_q2 * lambda_k2)
           + lambda_init).astype(np.float32)
    one_m_li = np.float32(1.0 - lambda_init)
    scale = np.float32(HEAD_DIM ** -0.5)

    xT = np.ascontiguousarray(x[0].T)
    W3 = W_qkv.reshape(3, N_HEADS, 128, D_MODEL)
    tri = (np.arange(512)[None, :128] >= np.arange(128)[:, None])
    trimask = np.ascontiguousarray(tri[:, :128]).astype(np.float32)
    onesin = np.ones((128, 128), np.float32)
    meanin = np.full((128, 1), 1.0 / 128, np.float32)

    in_maps = []
    for i in range(N_CORES):
        hs = [HPC * i + k for k in range(HPC)]
        wq = np.concatenate([W3[0, h] * scale for h in hs], 0)   # [256, D]
        wk = np.concatenate([W3[1, h] for h in hs], 0)           # [256, D]
        wv = np.concatenate([W3[2, h] for h in hs], 0)           # [256, D]
        wqkT = np.ascontiguousarray(np.concatenate([wq, wk], 0).T)
        wvT = np.ascontiguousarray(wv.T)
        woT = np.ascontiguousarray(W_o[:, 128 * hs[0]:128 * (hs[-1] + 1)].T)
        in_maps.append({
            "xT": xT,
            "wqkT": wqkT,
            "wvT": wvT,
            "woT": woT,
            "lamneg": np.ascontiguousarray(-lam),
            "gamma": np.ascontiguousarray(gn_gamma[hs] * one_m_li),
            "beta": np.ascontiguousarray(gn_beta[hs] * one_m_li),
            "trimask": trimask,
            "onesin": onesin,
            "meanin": meanin,
        })

    nc = _get_nc()
    trace = bool(int(os.environ.get("KERNEL_TRACE", "0")))
    res = run_bass_kernel_spmd(nc, in_maps, core_ids=list(range(N_CORES)),
                               trace=trace)
    if trace:
        _NC_CACHE["last_result"] = res
    y = np.zeros((SEQ, D_MODEL), np.float32)
    for r in res.results:
        y += np.asarray(r["y"], np.float32)
    return y[None]


# revision 25
# speedup vs baseline: 1.3422x; 1.3422x over previous
"""DifferentialAttention on 8 TRN2 NeuronCores.

Sharding: tensor-parallel over heads (2 heads per core), no device
collectives. Each core computes qkv for its heads, causal differential
attention + per-head LayerNorm, and a partial output projection through
its slice of W_o columns; the host sums the 8 partial outputs.

All matmuls run as float32r (fp22 mantissa, full PE rate at N>=256).
"""

import numpy as np

HEAD_DIM = 64
N_HEADS = 16
D_MODEL = 2048
SEQ = 2048
LAYER_IDX = 12
LN_EPS = 1e-5
N_CORES = 8
HPC = N_HEADS // N_CORES          # heads per core = 2
CHUNK = 512                       # sq chunk width
NCHUNK = SEQ // CHUNK             # 4
NDT = D_MODEL // 128              # 16 d-tiles
NST = SEQ // 128                  # 16 s-tiles

_SYNC_CNT = [0]


def _patch_tile_drain(tile_mod, bass_rust):
    """The walrus build in this container encodes at most one sem wait per
    instruction; TileContext's exit drain carries one wait per producer
    proc. Split the extras onto single-wait NOPs."""
    from concourse.vector_clock import ScopedClock

    def patched(self, tick_clock, wait_clock):
        nc = self.nc
        drain_inst = nc.sync.drain()
        wait_clock.add_sem_waits(
            drain_inst.ins, ScopedClock({None: tick_clock.global_clock})
        )
        si = drain_inst.ins.sync_info
        waits = list(si.on_wait or [])
        if len(waits) > 1:
            si.on_wait = [waits[0]]
            for w in waits[1:]:
                nop = nc.sync.nop()
                nop.ins.sync_info = bass_rust.SyncInfo(on_wait=[w], on_update=[])
        nc.all_engine_barrier()
        popped = nc._tile_sem_poison_stack.pop()
        assert popped is self._sem_poison
        nc.clear_and_free_semaphores(list(self.sems.allocated().values()))
        nc.all_engine_barrier()

    tile_mod.TileContext._drain_and_barrier = patched


def _fix_sync_limits(nc, mybir, bass_rust):
    """Split multi-wait / multi-update instructions into single-wait NOP
    chains on the same engine queue (walrus single-sync-slot limit)."""

    def nop(engine, wait=None, update=None):
        _SYNC_CNT[0] += 1
        n = mybir.InstNoOp(name=f"syncsplit-{_SYNC_CNT[0]}", ins=[], outs=[])
        n.engine = engine
        n.sync_info = bass_rust.SyncInfo(
            on_wait=[wait] if wait is not None else [],
            on_update=[update] if update is not None else [],
        )
        return n

    for f in nc.m.functions:
        for b in f.blocks:
            out = []
            for inst in b.instructions:
                si = inst.sync_info
                post = []
                if si is not None:
                    waits = list(si.on_wait or [])
                    if len(waits) > 1:
                        for w in waits[:-1]:
                            out.append(nop(inst.engine, wait=w))
                        si.on_wait = [waits[-1]]
                    ups = list(si.on_update or [])
                    if len(ups) > 1:
                        si.on_update = [ups[0]]
                        for u in ups[1:]:
                            post.append(nop(inst.engine, update=u))
                out.append(inst)
                out.extend(post)
            b.instructions = out


def _install_ntff_shim():
    """Register the axon NTFF profile hook (used only when tracing)."""
    import sys, types
    if "antenv.axon_hooks" in sys.modules:
        return
    try:
        mod = types.ModuleType("antenv.axon_hooks")
        mod._hook = None
        mod.set_axon_ntff_profile_hook = lambda h: setattr(mod, "_hook", h)
        mod.get_axon_ntff_profile_hook = lambda: mod._hook
        sys.modules["antenv.axon_hooks"] = mod
        import antenv
        antenv.axon_hooks = mod
        from trn_agent_boot.trn_boot import _ntff_profile_via_ctypes
        mod.set_axon_ntff_profile_hook(
            _ntff_profile_via_ctypes("/opt/axon/libaxon_pjrt.so")
        )
    except Exception:
        pass


def _build_nc():
    import bass_rust
    import concourse.bass as bass
    import concourse.tile as tile
    from concourse import mybir

    _patch_tile_drain(tile, bass_rust)

    f32 = mybir.dt.float32
    f32r = mybir.dt.float32r
    bf16 = mybir.dt.bfloat16
    AT = mybir.ActivationFunctionType
    OP = mybir.AluOpType

    nc = bass.Bass()

    xT = nc.dram_tensor("xT", [D_MODEL, SEQ], f32r, kind="ExternalInput")
    wqkT = nc.dram_tensor("wqkT", [D_MODEL, 4 * 128], f32r, kind="ExternalInput")
    wvT = nc.dram_tensor("wvT", [D_MODEL, HPC * 128], f32r, kind="ExternalInput")
    woT = nc.dram_tensor("woT", [HPC * 128, D_MODEL], f32r, kind="ExternalInput")
    lamneg = nc.dram_tensor("lamneg", [128], f32, kind="ExternalInput")
    gamma = nc.dram_tensor("gamma", [HPC, 128], f32, kind="ExternalInput")
    beta = nc.dram_tensor("beta", [HPC, 128], f32, kind="ExternalInput")
    trimask = nc.dram_tensor("trimask", [128, 128], f32r, kind="ExternalInput")
    onesin = nc.dram_tensor("onesin", [128, 128], f32r, kind="ExternalInput")
    meanin = nc.dram_tensor("meanin", [128, 1], f32r, kind="ExternalInput")
    y = nc.dram_tensor("y", [SEQ, D_MODEL], bf16, kind="ExternalOutput")

    with tile.TileContext(nc) as tc, nc.allow_low_precision(reason="fp32r pipeline"):
        import contextlib
        with contextlib.ExitStack() as ctx:
            consts = ctx.enter_context(tc.tile_pool(name="consts", bufs=1))
            main = ctx.enter_context(tc.tile_pool(name="main", bufs=1))
            drp = ctx.enter_context(tc.tile_pool(name="drp", bufs=4, space="DRAM"))

            # ---- constants ----
            lam_sb = consts.tile([128, 1], f32)
            nc.sync.dma_start(lam_sb[:, 0], lamneg[:])
            gam_sb = [consts.tile([128, 1], f32, name=f"gam{h}") for h in range(HPC)]
            bet_sb = [consts.tile([128, 1], f32, name=f"bet{h}") for h in range(HPC)]
            for h in range(HPC):
                nc.sync.dma_start(gam_sb[h][:, 0], gamma[h, :])
                nc.sync.dma_start(bet_sb[h][:, 0], beta[h, :])
            tri_sb = consts.tile([128, 128], f32r)
            nc.sync.dma_start(tri_sb[:], trimask[:])
            ones_col = consts.tile([128, 1], f32r)
            nc.sync.dma_start(ones_col[:], onesin[:, 0:1])
            mean_col = consts.tile([128, 1], f32r)
            nc.sync.dma_start(mean_col[:], meanin[:])

            # ---- persistent activations ----
            qk_sb = [main.tile([128, SEQ], f32r, name=f"qk{i}") for i in range(4)]
            v_sb = [main.tile([128, HPC * 128], f32r, name=f"v{t}") for t in range(NST)]
            w_sb = [main.tile([128, SEQ], f32r, name=f"w{h}") for h in range(HPC)]
            outT_sb = [main.tile([128, SEQ], f32r, name=f"outT{h}") for h in range(HPC)]
            wo_sb = [main.tile([128, SEQ], f32r, name=f"wo{i}") for i in range(HPC)]
            d1_all = [main.tile([1, SEQ], f32, name=f"d1a{h}") for h in range(HPC)]

            def bcast(vec_ap, out_ap, n):
                # [1, n] sbuf -> dram -> [128, n] sbuf partition-broadcast
                s = drp.tile([1, n], f32, tag="dbc")
                nc.sync.dma_start(s[:], vec_ap)
                bap = bass.AP(tensor=s.tensor, offset=s.offset,
                              ap=[[0, 128]] + list(s.ap[1:]))
                nc.sync.dma_start(out_ap, bap)

            # ================= phase 1: qkv projection =================
            with (
                tc.tile_pool(name="p1w", bufs=1) as p1w,
                tc.tile_pool(name="p1x", bufs=18) as p1x,
                tc.tile_pool(name="p1ps", bufs=4, space="PSUM") as p1ps,
            ):
                wqk_t = [p1w.tile([128, 512], f32r, name=f"wqk{d}") for d in range(NDT)]
                wv_t = [p1w.tile([128, HPC * 128], f32r, name=f"wv{d}") for d in range(NDT)]

                for c in range(NCHUNK):
                    xc = []
                    for d in range(NDT):
                        if c == 0:
                            nc.sync.dma_start(
                                wqk_t[d][:], wqkT[128 * d:128 * (d + 1), :])
                        t = p1x.tile([128, CHUNK], f32r, tag="xc")
                        nc.sync.dma_start(
                            t[:], xT[128 * d:128 * (d + 1), CHUNK * c:CHUNK * (c + 1)])
                        xc.append(t)
                        if c == 0:
                            nc.sync.dma_start(
                                wv_t[d][:], wvT[128 * d:128 * (d + 1), :])
                    if c == 1:
                        for i in range(HPC):
                            nc.sync.dma_start(
                                wo_sb[i][:], woT[128 * i:128 * (i + 1), :])
                    # d-outer accumulation: compute starts once the first
                    # d-tile of weights+x has landed
                    qps = [p1ps.tile([128, CHUNK], f32, tag="qkps", name=f"qps{ct}")
                           for ct in range(4)]
                    vps = [p1ps.tile([128, HPC * 128], f32, tag="vps", name=f"vps{ss}")
                           for ss in range(4)]
                    for d in range(NDT):
                        for ct in range(4):
                            nc.tensor.matmul(
                                qps[ct][:], wqk_t[d][:, 128 * ct:128 * (ct + 1)],
                                xc[d][:], start=(d == 0), stop=(d == NDT - 1))
                        for ss in range(4):
                            nc.tensor.matmul(
                                vps[ss][:], xc[d][:, 128 * ss:128 * (ss + 1)],
                                wv_t[d][:], start=(d == 0), stop=(d == NDT - 1))
                    for ct in range(4):
                        nc.vector.tensor_copy(
                            qk_sb[ct][:, CHUNK * c:CHUNK * (c + 1)], qps[ct][:])
                    for ss in range(4):
                        nc.vector.tensor_copy(v_sb[4 * c + ss][:], vps[ss][:])

            # ===== phase 2: differential attention =====
            # A/d psums drain to SBUF immediately at chunk end so
            # single-buffered accumulators release fast; 4 exp-psum slots
            # decouple the score matmuls from ACT.
            with (
                tc.tile_pool(name="p2e", bufs=4, space="PSUM") as p2e,
                tc.tile_pool(name="p2a1", bufs=1, space="PSUM") as p2a1,
                tc.tile_pool(name="p2a2", bufs=1, space="PSUM") as p2a2,
                tc.tile_pool(name="p2d1", bufs=1, space="PSUM") as p2d1,
                tc.tile_pool(name="p2d2", bufs=1, space="PSUM") as p2d2,
                tc.tile_pool(name="p2sb", bufs=3) as p2sb,
                tc.tile_pool(name="p2t", bufs=2) as p2t,
                tc.tile_pool(name="p2s", bufs=8) as p2s,
            ):
                for h in range(HPC):
                    qT = qk_sb[h]
                    kT = qk_sb[2 + h]
                    for c in range(NCHUNK):
                        n_sk = 4 * (c + 1)
                        csl = slice(CHUNK * c, CHUNK * (c + 1))
                        a1 = p2a1.tile([128, CHUNK], f32, tag="a1")
                        a2 = p2a2.tile([128, CHUNK], f32, tag="a2")
                        d1 = p2d1.tile([1, CHUNK], f32, tag="d1")
                        d2 = p2d2.tile([1, CHUNK], f32, tag="d2")
                        for t in range(n_sk):
                            diag = t >= 4 * c
                            f0 = 128 * (t - 4 * c) if diag else 0
                            sl = slice(f0, CHUNK)
                            qsl = slice(CHUNK * c + f0, CHUNK * (c + 1))
                            e1p = p2e.tile([128, CHUNK], f32, tag="e")
                            e2p = p2e.tile([128, CHUNK], f32, tag="e")
                            nc.tensor.matmul(
                                e1p[:, sl], kT[0:64, 128 * t:128 * (t + 1)],
                                qT[0:64, qsl], start=True, stop=True)
                            nc.tensor.matmul(
                                e2p[:, sl], kT[64:128, 128 * t:128 * (t + 1)],
                                qT[64:128, qsl], start=True, stop=True)
                            e1 = p2sb.tile([128, CHUNK], f32r, tag="e1")
                            e2 = p2sb.tile([128, CHUNK], f32r, tag="e2")
                            nc.scalar.activation(e1[:, sl], e1p[:, sl], AT.Exp)
                            nc.scalar.activation(e2[:, sl], e2p[:, sl], AT.Exp)
                            if diag:
                                dsl = slice(f0, f0 + 128)
                                nc.vector.tensor_tensor(
                                    e1[:, dsl], e1[:, dsl], tri_sb[:], OP.mult)
                                nc.vector.tensor_tensor(
                                    e2[:, dsl], e2[:, dsl], tri_sb[:], OP.mult)
                            first, last = (t == 0), (t == n_sk - 1)
                            vt = v_sb[t][:, 128 * h:128 * (h + 1)]
                            nc.tensor.matmul(a1[:, sl], vt, e1[:, sl],
                                             start=first, stop=last)
                            nc.tensor.matmul(a2[:, sl], vt, e2[:, sl],
                                             start=first, stop=last)
                            nc.tensor.matmul(d1[:, sl], ones_col[:], e1[:, sl],
                                             start=first, stop=last)
                            nc.tensor.matmul(d2[:, sl], ones_col[:], e2[:, sl],
                                             start=first, stop=last)
                        # drain psums to SBUF fast, then combine from SBUF:
                        # w' = A1u - (d1/d2)*lam*A2u   (LN scale-invariance)
                        a1s = p2t.tile([128, CHUNK], f32, tag="a1s")
                        a2s = p2t.tile([128, CHUNK], f32, tag="a2s")
                        nc.vector.tensor_copy(a1s[:], a1[:])
                        nc.vector.tensor_copy(a2s[:], a2[:])
                        nc.vector.tensor_copy(d1_all[h][:, csl], d1[:])
                        d2s = p2s.tile([1, CHUNK], f32, tag="rd")
                        nc.vector.tensor_copy(d2s[:], d2[:])
                        rd2 = p2s.tile([1, CHUNK], f32, tag="rd")
                        nc.vector.reciprocal(rd2[:], d2s[:])
                        r = p2s.tile([1, CHUNK], f32, tag="rd")
                        nc.vector.tensor_tensor(r[:], d1_all[h][:, csl], rd2[:],
                                                OP.mult)
                        rb = p2t.tile([128, CHUNK], f32, tag="rb")
                        bcast(r[:], rb[:], CHUNK)
                        ta2 = p2t.tile([128, CHUNK], f32, tag="ta2")
                        nc.vector.tensor_tensor(ta2[:], a2s[:], rb[:], OP.mult)
                        nc.vector.scalar_tensor_tensor(
                            w_sb[h][:, csl],
                            in0=ta2[:], scalar=lam_sb[:], in1=a1s[:],
                            op0=OP.mult, op1=OP.add)

            # ========== tail: per-chunk LayerNorm (both heads) + projection ==========
            with (
                tc.tile_pool(name="p3e", bufs=2, space="PSUM") as p3e,
                tc.tile_pool(name="p3y", bufs=4, space="PSUM") as p3y,
                tc.tile_pool(name="p3t", bufs=2) as p3t,
                tc.tile_pool(name="p3s", bufs=8) as p3s,
                tc.tile_pool(name="p3sb", bufs=4) as p3sb,
            ):
                for c in range(NCHUNK):
                    csl = slice(CHUNK * c, CHUNK * (c + 1))
                    for h in range(HPC):
                        wsq = p3t.tile([128, CHUNK], f32r, tag="wsq")
                        nc.vector.tensor_tensor(wsq[:], w_sb[h][:, csl],
                                                w_sb[h][:, csl], OP.mult)
                        s1f = p3e.tile([128, CHUNK], f32, tag="e", name="s1f")
                        s2f = p3e.tile([128, CHUNK], f32, tag="e", name="s2f")
                        s1 = s1f[0:1, :]
                        s2 = s2f[0:1, :]
                        nc.tensor.matmul(s1, mean_col[:], w_sb[h][:, csl],
                                         start=True, stop=True)
                        nc.tensor.matmul(s2, mean_col[:], wsq[:],
                                         start=True, stop=True)
                        mu = p3s.tile([1, CHUNK], f32, tag="sm")
                        nc.vector.tensor_copy(mu[:], s1)
                        mu2 = p3s.tile([1, CHUNK], f32, tag="sm")
                        nc.vector.tensor_tensor(mu2[:], mu[:], mu[:], OP.mult)
                        var = p3s.tile([1, CHUNK], f32, tag="sm")
                        nc.vector.tensor_tensor(var[:], s2, mu2[:], OP.subtract)
                        d1sq = p3s.tile([1, CHUNK], f32, tag="sm")
                        nc.vector.tensor_tensor(d1sq[:], d1_all[h][:, csl],
                                                d1_all[h][:, csl], OP.mult)
                        varep = p3s.tile([1, CHUNK], f32, tag="sm")
                        nc.vector.scalar_tensor_tensor(
                            varep[:], in0=d1sq[:], scalar=LN_EPS, in1=var[:],
                            op0=OP.mult, op1=OP.add)
                        # rsqrt via exp(-0.5*ln(x)) — same ACT table set as exp
                        lnv = p3s.tile([1, CHUNK], f32, tag="sm")
                        nc.scalar.activation(lnv[:], varep[:], AT.Ln)
                        rsd = p3s.tile([1, CHUNK], f32, tag="sm")
                        nc.scalar.activation(rsd[:], lnv[:], AT.Exp, scale=-0.5)
                        mrs = p3s.tile([1, CHUNK], f32, tag="sm")
                        nc.vector.tensor_tensor(mrs[:], mu[:], rsd[:], OP.mult)
                        rsd_b = p3t.tile([128, CHUNK], f32, tag="rsdb")
                        bcast(rsd[:], rsd_b[:], CHUNK)
                        mrs_b = p3t.tile([128, CHUNK], f32, tag="mrsb")
                        bcast(mrs[:], mrs_b[:], CHUNK)
                        u1 = p3t.tile([128, CHUNK], f32, tag="u1")
                        nc.vector.tensor_tensor(u1[:], w_sb[h][:, csl], rsd_b[:],
                                                OP.mult)
                        u2 = p3t.tile([128, CHUNK], f32, tag="u2")
                        nc.vector.tensor_tensor(u2[:], u1[:], mrs_b[:],
                                                OP.subtract)
                        nc.vector.tensor_scalar(
                            outT_sb[h][:, csl], u2[:], gam_sb[h][:], bet_sb[h][:],
                            OP.mult, OP.add)
                    for st in range(4 * c, 4 * (c + 1)):
                        ssl = slice(128 * st, 128 * (st + 1))
                        for oc in range(NCHUNK):
                            osl = slice(CHUNK * oc, CHUNK * (oc + 1))
                            yp = p3y.tile([128, CHUNK], f32, tag="y")
                            for i in range(HPC):
                                nc.tensor.matmul(
                                    yp[:], outT_sb[i][:, ssl], wo_sb[i][:, osl],
                                    start=(i == 0), stop=(i == HPC - 1))
                            ys = p3sb.tile([128, CHUNK], bf16, tag="ys")
                            if (st + oc) % 2 == 0:
                                nc.vector.tensor_copy(ys[:], yp[:])
                            else:
                                nc.scalar.copy(ys[:], yp[:])
                            nc.sync.dma_start(y[ssl, osl], ys[:])

    from concourse import mybir as _mb
    _fix_sync_limits(nc, _mb, bass_rust)
    return nc


_NC_CACHE = {}


def _get_nc():
    if "nc" not in _NC_CACHE:
        _NC_CACHE["nc"] = _build_nc()
    return _NC_CACHE["nc"]


def kernel(x, W_qkv, W_o, lambda_q1, lambda_k1, lambda_q2, lambda_k2,
           gn_gamma, gn_beta):
    import os
    _install_ntff_shim()
    from concourse.bass_utils import run_bass_kernel_spmd

    x = np.asarray(x, np.float32)
    W_qkv = np.asarray(W_qkv, np.float32)
    W_o = np.asarray(W_o, np.float32)
    lambda_q1 = np.asarray(lambda_q1, np.float32)
    lambda_k1 = np.asarray(lambda_k1, np.float32)
    lambda_q2 = np.asarray(lambda_q2, np.float32)
    lambda_k2 = np.asarray(lambda_k2, np.float32)
    gn_gamma = np.asarray(gn_gamma, np.float32)
    gn_beta = np.asarray(gn_beta, np.float32)

    lambda_init = np.float32(0.8 - 0.6 * np.exp(-0.3 * LAYER_IDX))
    lam = (np.exp(lambda_q1 * lambda_k1) - np.exp(lambda_q2 * lambda_k2)
           + lambda_init).astype(np.float32)
    one_m_li = np.float32(1.0 - lambda_init)
    scale = np.float32(HEAD_DIM ** -0.5)

    xT = np.ascontiguousarray(x[0].T)
    W3 = W_qkv.reshape(3, N_HEADS, 128, D_MODEL)
    tri = (np.arange(512)[None, :128] >= np.arange(128)[:, None])
    trimask = np.ascontiguousarray(tri[:, :128]).astype(np.float32)
    onesin = np.ones((128, 128), np.float32)
    meanin = np.full((128, 1), 1.0 / 128, np.float32)

    in_maps = []
    for i in range(N_CORES):
        hs = [HPC * i + k for k in range(HPC)]
        wq = np.concatenate([W3[0, h] * scale for h in hs], 0)   # [256, D]
        wk = np.concatenate([W3[1, h] for h in hs], 0)           # [256, D]
        wv = np.concatenate([W3[2, h] for h in hs], 0)           # [256, D]
        wqkT = np.ascontiguousarray(np.concatenate([wq, wk], 0).T)
        wvT = np.ascontiguousarray(wv.T)
        woT = np.ascontiguousarray(W_o[:, 128 * hs[0]:128 * (hs[-1] + 1)].T)
        in_maps.append({
            "xT": xT,
            "wqkT": wqkT,
            "wvT": wvT,
            "woT": woT,
            "lamneg": np.ascontiguousarray(-lam),
            "gamma": np.ascontiguousarray(gn_gamma[hs] * one_m_li),
            "beta": np.ascontiguousarray(gn_beta[hs] * one_m_li),
            "trimask": trimask,
            "onesin": onesin,
            "meanin": meanin,
        })

    nc = _get_nc()
    trace = bool(int(os.environ.get("KERNEL_TRACE", "0")))
    res = run_bass_kernel_spmd(nc, in_maps, core_ids=list(range(N_CORES)),
                               trace=trace)
    if trace:
        _NC_CACHE["last_result"] = res
    y = np.zeros((SEQ, D_MODEL), np.float32)
    for r in res.results:
        y += np.asarray(r["y"], np.float32)
    return y[None]
